# revision 16
# baseline (speedup 1.0000x reference)
# Trainium2 Bass kernel for AttentionWithSink
# B=2, S=2048, D=1024, H=16 heads (hd=64), 8 sink tokens, full bidirectional
# attention over T=2056 tokens, output projection back to D.
#
# Sharding: 8 cores = 2 batches x 4 head-groups (4 heads each).
# Each core computes QKV for its 4 heads over its batch, transposed-scores
# attention (keys on partitions => no transposes anywhere), and a partial
# output projection over its 256 head-dims. Host sums the 4 partials per
# batch (tensor-parallel unshard) and adds the bv/bo bias contribution.
#
# v3: fp16 operands end-to-end (PSUM accumulation stays fp32).
# - exp split ACT (AF.Exp) / DVE (Schraudolph int16 bitcast) by key chunk
# - no bias matmuls: bo and bv folded into a host-side constant vector
#   (softmax weights sum to 1, so  out = attn@wo.T + (bv@wo.T + bo))
# - softmax denominators: reciprocal_approx_fast directly on the PSUM row
# - input DMAs in consumption order (512-col x slices, weights interleaved)
# - y output in fp16, drains rotated ACT/DVE
import numpy as np

B, S, D, H, HD, NS = 2, 2048, 1024, 16, 64, 8
T = S + NS            # 2056 tokens incl. sinks (sinks stored LAST)
NCORES = 8
HPG = 4               # heads per group/core
GD = HPG * HD         # 256 head-dims per core
NKC = 17              # key chunks: 16*128 + 8
KREM = T - 16 * 128   # 8
NQC = 4               # query chunks
QCH = 512
VBLK = 130            # v' block per chunk: vA(64)|1|vB(64)|1

# Schraudolph fp16 exp: i16 = in*SCHRAU_A + SCHRAU_B, bitcast to fp16.
SCHRAU_A = float(2**10 / np.log(2.0))
SCHRAU_C = 0.043677448 * 2**10
SCHRAU_B = float(15 * 2**10 - SCHRAU_C)
# Each chunk's exp is split into two parallel instructions: ACT (exact exp)
# on AW columns, DVE (Schraudolph) on the rest; sides alternate per chunk so
# the Schraudolph error spreads evenly over queries.
AW = 576

_prog_cache = {}


def _emit_body(nc, tc, tile, mybir, dr, pers):
    F32 = mybir.dt.float32
    F16 = mybir.dt.float16
    I16 = mybir.dt.int16
    AF = mybir.ActivationFunctionType
    ALU = mybir.AluOpType
    qT, kT, vp, wo_sb, bqt, bkt = pers

    with (
        tc.tile_pool(name="xw", bufs=1) as xw,
        tc.tile_pool(name="ps", bufs=2, space="PSUM") as psb,
        tc.tile_pool(name="pt", bufs=6) as ptp,
        tc.tile_pool(name="ysb", bufs=6) as ysp,
        tc.tile_pool(name="small", bufs=4) as sp,
    ):
        scp = avp = pyp = psv = psb
        # ---------------- input DMAs (consumption order) ----------------
        xt = [xw.tile([128, T], F16, tag=f"x{dd}", name=f"x{dd}") for dd in range(8)]
        w_sb = {
            nm: xw.tile([128, 8 * GD], F16, tag=f"w{nm}", name=f"w{nm}")
            for nm in ("k", "q", "v")
        }
        # sync has a hardware DGE and no other early work: it issues everything
        # needed first (w_k, x slices 0-1, w_q). scalar (hardware DGE, but its
        # stream starts with the ~1.3us ACT table load) takes slice 2 + w_v.
        # gpsimd descriptor-gen is ~650ns per DMA (software DGE): it only gets
        # the late-needed slice 3 + w_o.
        nc.sync.dma_start(w_sb["k"][:], dr["wk_p"][:])
        def _x_dma(eng, si_c0, si_c1, dd):
            eng.dma_start(
                xt[dd][:, si_c0:si_c1], dr["xT"][dd * 128 : (dd + 1) * 128, si_c0:si_c1]
            )
        for dd in range(8):
            _x_dma(nc.sync, 0, 512, dd)
        nc.sync.dma_start(w_sb["q"][:], dr["wq_p"][:])
        for dd in range(8):
            _x_dma(nc.sync, 512, 1024, dd)
        for dd in range(8):
            _x_dma(nc.scalar, 1024, 1536, dd)
        nc.scalar.dma_start(w_sb["v"][:], dr["wv_p"][:])
        for dd in range(8):
            _x_dma(nc.gpsimd, 1536, T, dd)
        for i in range(2):
            nc.gpsimd.dma_start(wo_sb[i][:], dr["wo_t"][i * 128 : (i + 1) * 128, :])

        # ---------------- projections ----------------
        def emit_k(kc5s):
            # kT[i][gd, tok] = wk_i.T @ x ; bias+drain on ACT
            for kc5 in kc5s:
                for i in range(2):
                    n = QCH if kc5 < 4 else KREM
                    pk = psb.tile([128, QCH], F32, tag="pq", name=f"pk{i}_{kc5}")
                    for dd in range(8):
                        nc.tensor.matmul(
                            pk[:, :n],
                            w_sb["k"][:, dd * GD + i * 128 : dd * GD + i * 128 + 128],
                            xt[dd][:, kc5 * QCH : kc5 * QCH + n],
                            start=(dd == 0),
                            stop=(dd == 7),
                        )
                    nc.scalar.activation(
                        kT[i][:, kc5 * QCH : kc5 * QCH + n], pk[:, :n],
                        AF.Identity, bias=bkt[i][:, 0:1],
                    )

        def q_group(qc, i):
            pq = psb.tile([128, QCH], F32, tag="pq", name=f"pq{i}_{qc}")
            for dd in range(8):
                nc.tensor.matmul(
                    pq[:],
                    w_sb["q"][:, dd * GD + i * 128 : dd * GD + i * 128 + 128],
                    xt[dd][:, qc * QCH : (qc + 1) * QCH],
                    start=(dd == 0),
                    stop=(dd == 7),
                )
            nc.scalar.activation(
                qT[i][:, qc * QCH : (qc + 1) * QCH], pq[:],
                AF.Identity, bias=bqt[i][:, 0:1],
            )

        def emit_q(qcs):
            for qc in qcs:
                for i in range(2):
                    q_group(qc, i)

        def emit_v_chunk(tcx):
            kk = 128 if tcx < 16 else KREM
            pvt = psv.tile([128, QCH], F32, tag="pq", name=f"pv{tcx}")
            pv = pvt[:, 0:GD]
            for dd in range(8):
                nc.tensor.matmul(
                    pv[:kk, :],
                    xt[dd][:, tcx * 128 : tcx * 128 + kk],
                    w_sb["v"][:, dd * GD : (dd + 1) * GD],
                    start=(dd == 0),
                    stop=(dd == 7),
                )
            c0 = tcx * VBLK
            for pr in range(2):
                for hh in range(2):
                    h = pr * 2 + hh
                    src = pv[:kk, h * 64 : (h + 1) * 64]
                    dst = vp[pr][:kk, c0 + hh * 65 : c0 + hh * 65 + 64]
                    if (tcx + pr) % 2 == 0:
                        nc.scalar.activation(dst, src, AF.Identity)
                    else:
                        nc.vector.tensor_copy(dst, src)

        # ---------------- out-projection ----------------
        def y_unit(qc, onTs, ts_, dc):
            py = pyp.tile([128, QCH], F32, tag="pq", name=f"py_{qc}_{ts_}_{dc}")
            for pr in range(2):
                nc.tensor.matmul(
                    py[:],
                    onTs[pr][:, ts_ * 128 : (ts_ + 1) * 128],
                    wo_sb[pr][:, dc * QCH : (dc + 1) * QCH],
                    start=(pr == 0), stop=(pr == 1),
                )
            ys = ysp.tile([128, QCH], F16, tag="ys", name=f"ys_{qc}_{ts_}_{dc}")
            if (ts_ + dc) % 2 == 0:
                nc.scalar.activation(ys[:], py[:], AF.Identity)
            else:
                nc.vector.tensor_copy(ys[:], py[:])
            nc.sync.dma_start(
                dr["y"][qc * QCH + ts_ * 128 : qc * QCH + (ts_ + 1) * 128,
                        dc * QCH : (dc + 1) * QCH],
                ys[:],
            )

        def emit_y(qc, onTs):
            for ts_ in range(4):
                for dc in range(2):
                    y_unit(qc, onTs, ts_, dc)

        # ---------------- attention ----------------
        def att_unit(qc, with_v, fillers=()):
            # fillers: independent work closures (y-proj of the previous query
            # chunk, q-proj groups) emitted INSIDE the chunk loop so the PE
            # FIFO always has ready matmuls ahead of dependency stalls.
            fillers = list(fillers)
            fi = 0
            onTs = []
            for pr in range(2):
                VA = avp.tile([128, QCH], F32, tag="av", name=f"VA_{qc}_{pr}")
                VB = avp.tile([128, QCH], F32, tag="av", name=f"VB_{qc}_{pr}")

                def emit_pv(kc, kk, PT):
                    st, stp = kc == 0, kc == NKC - 1
                    c0 = kc * VBLK
                    nc.tensor.matmul(
                        VA[0:65, :], vp[pr][:kk, c0 : c0 + 65],
                        PT[:kk, 0:QCH], start=st, stop=stp,
                    )
                    nc.tensor.matmul(
                        VB[0:65, :], vp[pr][:kk, c0 + 65 : c0 + VBLK],
                        PT[:kk, QCH : 2 * QCH], start=st, stop=stp,
                    )

                pend = []  # software pipeline: PV runs two chunks behind
                for kc in range(NKC):
                    if with_v and pr == 0:
                        emit_v_chunk(kc)
                    if kc % 2 == 1 and fi < len(fillers):
                        fillers[fi]()
                        fi += 1
                    kk = 128 if kc < 16 else KREM
                    Sps = scp.tile([128, 2 * QCH], F32, tag="s", name=f"S_{qc}_{pr}_{kc}")
                    PT = ptp.tile([128, 2 * QCH], F16, tag="pt", name=f"PT_{qc}_{pr}_{kc}")
                    nc.tensor.matmul(
                        Sps[:kk, 0:QCH],
                        kT[pr][0:64, kc * 128 : kc * 128 + kk],
                        qT[pr][0:64, qc * QCH : (qc + 1) * QCH],
                        start=True, stop=True,
                    )
                    nc.tensor.matmul(
                        Sps[:kk, QCH : 2 * QCH],
                        kT[pr][64:128, kc * 128 : kc * 128 + kk],
                        qT[pr][64:128, qc * QCH : (qc + 1) * QCH],
                        start=True, stop=True,
                    )
                    # exp split across both engines in parallel
                    if kc % 2 == 0:
                        a0, a1, d0, d1 = 0, AW, AW, 2 * QCH
                    else:
                        d0, d1, a0, a1 = 0, 2 * QCH - AW, 2 * QCH - AW, 2 * QCH
                    nc.scalar.activation(PT[:kk, a0:a1], Sps[:kk, a0:a1], AF.Exp)
                    with nc.allow_low_precision(reason="fp16 softmax weights"):
                        nc.vector.tensor_scalar(
                            PT[:kk, d0:d1].bitcast(I16),
                            Sps[:kk, d0:d1],
                            SCHRAU_A, SCHRAU_B,
                            op0=ALU.mult, op1=ALU.add,
                        )
                    pend.append((kc, kk, PT))
                    if len(pend) > 2:
                        emit_pv(*pend.pop(0))
                for p in pend:
                    emit_pv(*p)
                onT = ptp.tile([128, QCH], F16, tag="onT", name=f"onT_{qc}_{pr}")
                for hh, V in ((0, VA), (1, VB)):
                    rc = sp.tile([1, QCH], F32, tag="rc", name=f"rc_{qc}_{pr}_{hh}")
                    with nc.allow_low_precision(reason="softmax denom reciprocal"):
                        # reciprocal_approx_fast mis-reads PSUM: stage via SBUF
                        dn = sp.tile([1, QCH], F32, tag="dn", name=f"dn_{qc}_{pr}_{hh}")
                        nc.scalar.activation(dn[:], V[64:65, :], AF.Identity)
                        nc.vector.reciprocal_approx_fast(rc[:], dn[:])
                    bc = sp.tile([64, QCH], F32, tag="bc", name=f"bc_{qc}_{pr}_{hh}")
                    nc.gpsimd.partition_broadcast(bc[:], rc[:])
                    with nc.allow_low_precision(reason="fp16 attn output"):
                        nc.vector.tensor_tensor(
                            onT[hh * 64 : hh * 64 + 64, :],
                            V[0:64, :], bc[:], op=ALU.mult,
                        )
                onTs.append(onT)
            while fi < len(fillers):
                fillers[fi]()
                fi += 1
            return onTs

        emit_k([0, 1])
        emit_q([0, 1])
        emit_k([2, 3, 4])
        y_pend = None
        for qc in range(NQC):
            fillers = []
            if y_pend is not None:
                pqc, ponTs = y_pend
                for ts_ in range(4):
                    for dc in range(2):
                        fillers.append(
                            lambda ts_=ts_, dc=dc, pqc=pqc, ponTs=ponTs: y_unit(
                                pqc, ponTs, ts_, dc
                            )
                        )
            if qc == 1:
                for qq in (2, 3):
                    for i in range(2):
                        fillers.append(lambda qq=qq, i=i: q_group(qq, i))
            onTs = att_unit(qc, with_v=(qc == 0), fillers=fillers)
            y_pend = (qc, onTs)
        emit_y(*y_pend)


def _build_program(reps=1):
    import concourse.bass as bass  # noqa: F401
    import concourse.mybir as mybir
    import concourse.tile as tile
    from concourse import bacc

    F32 = mybir.dt.float32
    F16 = mybir.dt.float16

    nc = bacc.Bacc("TRN2", num_devices=NCORES)
    dr = {
        "xT": nc.dram_tensor("xT", [D, T], F16, kind="ExternalInput"),
        "wq_p": nc.dram_tensor("wq_p", [128, 8 * GD], F16, kind="ExternalInput"),
        "wk_p": nc.dram_tensor("wk_p", [128, 8 * GD], F16, kind="ExternalInput"),
        "wv_p": nc.dram_tensor("wv_p", [128, 8 * GD], F16, kind="ExternalInput"),
        "wo_t": nc.dram_tensor("wo_t", [GD, D], F16, kind="ExternalInput"),
        "bq": nc.dram_tensor("bq", [GD, 1], F32, kind="ExternalInput"),
        "bk": nc.dram_tensor("bk", [GD, 1], F32, kind="ExternalInput"),
        "y": nc.dram_tensor("y", [S, D], F16, kind="ExternalOutput"),
    }

    with tile.TileContext(nc) as tc:
        with tc.tile_pool(name="persist", bufs=1) as pp:
            qT = [pp.tile([128, S], F16, tag=f"qT{i}", name=f"qT{i}") for i in range(2)]
            kT = [pp.tile([128, T], F16, tag=f"kT{i}", name=f"kT{i}") for i in range(2)]
            vp = [
                pp.tile([128, NKC * VBLK], F16, tag=f"vp{i}", name=f"vp{i}")
                for i in range(2)
            ]
            wo_sb = [pp.tile([128, D], F16, tag=f"wo{i}", name=f"wo{i}") for i in range(2)]
            bqt = [pp.tile([128, 1], F32, tag=f"bq{i}", name=f"bq{i}") for i in range(2)]
            bkt = [pp.tile([128, 1], F32, tag=f"bk{i}", name=f"bk{i}") for i in range(2)]
            for i in range(2):
                nc.scalar.dma_start(bqt[i][:], dr["bq"][i * 128 : (i + 1) * 128, :])
                nc.scalar.dma_start(bkt[i][:], dr["bk"][i * 128 : (i + 1) * 128, :])
            # ones columns of v' (denominator rows of the PV matmuls)
            for i in range(2):
                nc.vector.memset(vp[i][:], 1.0)
            pers = (qT, kT, vp, wo_sb, bqt, bkt)
            for _rep in range(reps):
                _emit_body(nc, tc, tile, mybir, dr, pers)
    nc.compile()
    return nc


def _get_program(reps=1):
    key = f"nc{reps}_v3"
    if key not in _prog_cache:
        _prog_cache[key] = _build_program(reps)
    return _prog_cache[key]


def _pack(a):
    # [1024, 256] -> [128, 2048]: dd-th 128-row block becomes column block dd
    return np.concatenate([a[dd * 128 : (dd + 1) * 128] for dd in range(8)], axis=1)


def _host_inputs(x, sink_tokens, wq, bq, wk, bk, wv, bv, wo, bo):
    f = np.float32
    h = np.float16
    x = np.asarray(x, f)
    sink = np.asarray(sink_tokens, f)[0]            # [NS, D]
    wq, wk, wv, wo = (np.asarray(a, f) for a in (wq, wk, wv, wo))
    bq, bk = (np.asarray(a, f) for a in (bq, bk))
    sc = np.float32(1.0 / np.sqrt(HD))
    in_maps = []
    for core in range(NCORES):
        b, g = core // 4, core % 4
        xs = np.concatenate([x[b], sink], axis=0)   # sinks LAST
        xT = np.ascontiguousarray(xs.T).astype(h)
        sl = slice(g * GD, (g + 1) * GD)
        in_maps.append({
            "xT": xT,
            "wq_p": _pack(np.ascontiguousarray(wq[sl].T) * sc).astype(h),
            "wk_p": _pack(np.ascontiguousarray(wk[sl].T)).astype(h),
            "wv_p": _pack(np.ascontiguousarray(wv[sl].T)).astype(h),
            "wo_t": np.ascontiguousarray(wo[:, sl].T).astype(h),
            "bq": (bq[sl] * sc).reshape(GD, 1).copy(),
            "bk": bk[sl].reshape(GD, 1).copy(),
        })
    return in_maps


def kernel(x, sink_tokens, wq, bq, wk, bk, wv, bv, wo, bo):
    from concourse.bass_utils import run_bass_kernel_spmd

    nc = _get_program()
    in_maps = _host_inputs(x, sink_tokens, wq, bq, wk, bk, wv, bv, wo, bo)
    res = None
    last_exc = None
    for attempt in range(3):
        try:
            res = run_bass_kernel_spmd(nc, in_maps, core_ids=list(range(NCORES)))
            break
        except Exception as e:  # transient NRT/axon failures: retry
            last_exc = e
            import time as _time
            _time.sleep(2.0 * (attempt + 1))
    if res is None:
        raise last_exc
    # host unshard: sum the 4 head-group partials per batch, then add the
    # bias constant (attn weights sum to 1 => bv passes straight through)
    bv = np.asarray(bv, np.float64)
    bo = np.asarray(bo, np.float64)
    wo64 = np.asarray(wo, np.float64)
    c = bv @ wo64.T + bo                            # [D]
    y = np.zeros((B, S, D), np.float64)
    for core in range(NCORES):
        y[core // 4] += res.results[core]["y"].astype(np.float64)
    y += c
    return y.astype(np.float32)


# revision 21
# speedup vs baseline: 1.1346x; 1.1346x over previous
# Trainium2 Bass kernel for AttentionWithSink
# B=2, S=2048, D=1024, H=16 heads (hd=64), 8 sink tokens, full bidirectional
# attention over T=2056 tokens, output projection back to D.
#
# Sharding: 8 cores = 2 batches x 4 head-groups (4 heads each).
# Each core computes QKV for its 4 heads over its batch, transposed-scores
# attention (keys on partitions => no transposes anywhere), and a partial
# output projection over its 256 head-dims. Host sums the 4 partials per
# batch (tensor-parallel unshard) and adds the bv/bo bias contribution.
#
# v3: fp16 operands end-to-end (PSUM accumulation stays fp32).
# - exp split ACT (AF.Exp) / DVE (Schraudolph int16 bitcast) by key chunk
# - no bias matmuls: bo and bv folded into a host-side constant vector
#   (softmax weights sum to 1, so  out = attn@wo.T + (bv@wo.T + bo))
# - softmax denominators: reciprocal_approx_fast directly on the PSUM row
# - input DMAs in consumption order (512-col x slices, weights interleaved)
# - y output in fp16, drains rotated ACT/DVE
import numpy as np

B, S, D, H, HD, NS = 2, 2048, 1024, 16, 64, 8
T = S + NS            # 2056 tokens incl. sinks (sinks stored LAST)
NCORES = 8
HPG = 4               # heads per group/core
GD = HPG * HD         # 256 head-dims per core
NKC = 17              # key chunks: 16*128 + 8
KREM = T - 16 * 128   # 8
NQC = 4               # query chunks
QCH = 512
VBLK = 130            # v' block per chunk: vA(64)|1|vB(64)|1

# Schraudolph fp16 exp: i16 = in*SCHRAU_A + SCHRAU_B, bitcast to fp16.
SCHRAU_A = float(2**10 / np.log(2.0))
SCHRAU_C = 0.043677448 * 2**10
SCHRAU_B = float(15 * 2**10 - SCHRAU_C)
# Each chunk's exp is split into two parallel instructions: one head on ACT
# (exact exp), the other on DVE (Schraudolph); heads swap engines per chunk
# so the Schraudolph error spreads evenly over queries.

_prog_cache = {}


def _emit_body(nc, tc, tile, mybir, dr, pers):
    F32 = mybir.dt.float32
    F16 = mybir.dt.float16
    I16 = mybir.dt.int16
    AF = mybir.ActivationFunctionType
    ALU = mybir.AluOpType
    qT, kT, vp, wo_sb, bqt, bkt = pers

    with (
        tc.tile_pool(name="xw", bufs=1) as xw,
        tc.tile_pool(name="ps", bufs=2, space="PSUM") as psb,
        tc.tile_pool(name="pt", bufs=6) as ptp,
        tc.tile_pool(name="ysb", bufs=6) as ysp,
        tc.tile_pool(name="small", bufs=4) as sp,
    ):
        scp = avp = pyp = psv = psb
        # ---------------- input DMAs (consumption order) ----------------
        xt = [xw.tile([128, T], F16, tag=f"x{dd}", name=f"x{dd}") for dd in range(8)]
        w_sb = {
            nm: xw.tile([128, 8 * GD], F16, tag=f"w{nm}", name=f"w{nm}")
            for nm in ("k", "q", "v")
        }
        # sync has a hardware DGE and no other early work: it issues everything
        # needed first (w_k, x slices 0-1, w_q). scalar (hardware DGE, but its
        # stream starts with the ~1.3us ACT table load) takes slice 2 + w_v.
        # gpsimd descriptor-gen is ~650ns per DMA (software DGE): it only gets
        # the late-needed slice 3 + w_o.
        nc.sync.dma_start(w_sb["k"][:], dr["wk_p"][:])
        def _x_dma(eng, si_c0, si_c1, dd):
            eng.dma_start(
                xt[dd][:, si_c0:si_c1], dr["xT"][dd * 128 : (dd + 1) * 128, si_c0:si_c1]
            )
        for dd in range(8):
            _x_dma(nc.sync, 0, 512, dd)
        nc.sync.dma_start(w_sb["q"][:], dr["wq_p"][:])
        for dd in range(8):
            _x_dma(nc.sync, 512, 1024, dd)
        for dd in range(8):
            _x_dma(nc.scalar, 1024, 1536, dd)
        nc.scalar.dma_start(w_sb["v"][:], dr["wv_p"][:])
        for dd in range(8):
            _x_dma(nc.gpsimd, 1536, T, dd)
        for i in range(2):
            nc.gpsimd.dma_start(wo_sb[i][:], dr["wo_t"][i * 128 : (i + 1) * 128, :])

        # ---------------- projections ----------------
        def emit_k(kc5s):
            # kT[i][gd, tok] = wk_i.T @ x ; bias+drain on ACT
            for kc5 in kc5s:
                for i in range(2):
                    n = QCH if kc5 < 4 else KREM
                    pk = psb.tile([128, QCH], F32, tag="pq", name=f"pk{i}_{kc5}")
                    for dd in range(8):
                        nc.tensor.matmul(
                            pk[:, :n],
                            w_sb["k"][:, dd * GD + i * 128 : dd * GD + i * 128 + 128],
                            xt[dd][:, kc5 * QCH : kc5 * QCH + n],
                            start=(dd == 0),
                            stop=(dd == 7),
                        )
                    nc.scalar.activation(
                        kT[i][:, kc5 * QCH : kc5 * QCH + n], pk[:, :n],
                        AF.Identity, bias=bkt[i][:, 0:1],
                    )

        def q_group(qc, i):
            pq = psb.tile([128, QCH], F32, tag="pq", name=f"pq{i}_{qc}")
            for dd in range(8):
                nc.tensor.matmul(
                    pq[:],
                    w_sb["q"][:, dd * GD + i * 128 : dd * GD + i * 128 + 128],
                    xt[dd][:, qc * QCH : (qc + 1) * QCH],
                    start=(dd == 0),
                    stop=(dd == 7),
                )
            nc.scalar.activation(
                qT[i][:, qc * QCH : (qc + 1) * QCH], pq[:],
                AF.Identity, bias=bqt[i][:, 0:1],
            )

        def emit_q(qcs):
            for qc in qcs:
                for i in range(2):
                    q_group(qc, i)

        def emit_v_chunk(tcx):
            kk = 128 if tcx < 16 else KREM
            pvt = psv.tile([128, QCH], F32, tag="pq", name=f"pv{tcx}")
            pv = pvt[:, 0:GD]
            for dd in range(8):
                nc.tensor.matmul(
                    pv[:kk, :],
                    xt[dd][:, tcx * 128 : tcx * 128 + kk],
                    w_sb["v"][:, dd * GD : (dd + 1) * GD],
                    start=(dd == 0),
                    stop=(dd == 7),
                )
            c0 = tcx * VBLK
            for pr in range(2):
                for hh in range(2):
                    h = pr * 2 + hh
                    src = pv[:kk, h * 64 : (h + 1) * 64]
                    dst = vp[pr][:kk, c0 + hh * 65 : c0 + hh * 65 + 64]
                    if (tcx + pr) % 2 == 0:
                        nc.scalar.activation(dst, src, AF.Identity)
                    else:
                        nc.vector.tensor_copy(dst, src)

        # ---------------- out-projection ----------------
        # each y unit is split into two single-matmul closures so the PE FIFO
        # can interleave them between attention chunks at fine grain
        def y_parts(qc, onTs, ts_, dc):
            py = pyp.tile([128, QCH], F32, tag="pq", name=f"py_{qc}_{ts_}_{dc}")

            def part0():
                nc.tensor.matmul(
                    py[:],
                    onTs[0][:, ts_ * 128 : (ts_ + 1) * 128],
                    wo_sb[0][:, dc * QCH : (dc + 1) * QCH],
                    start=True, stop=False,
                )

            def part1():
                nc.tensor.matmul(
                    py[:],
                    onTs[1][:, ts_ * 128 : (ts_ + 1) * 128],
                    wo_sb[1][:, dc * QCH : (dc + 1) * QCH],
                    start=False, stop=True,
                )
                ys = ysp.tile([128, QCH], F16, tag="ys", name=f"ys_{qc}_{ts_}_{dc}")
                if (ts_ + dc) % 2 == 0:
                    nc.scalar.activation(ys[:], py[:], AF.Identity)
                else:
                    nc.vector.tensor_copy(ys[:], py[:])
                nc.sync.dma_start(
                    dr["y"][qc * QCH + ts_ * 128 : qc * QCH + (ts_ + 1) * 128,
                            dc * QCH : (dc + 1) * QCH],
                    ys[:],
                )

            return [part0, part1]

        def emit_y(qc, onTs):
            for ts_ in range(4):
                for dc in range(2):
                    for p in y_parts(qc, onTs, ts_, dc):
                        p()

        # ---------------- attention ----------------
        def att_unit(qc, with_v, fillers=()):
            # fillers: independent work closures (y-proj of the previous query
            # chunk, q-proj groups) emitted INSIDE the chunk loop so the PE
            # FIFO always has ready matmuls ahead of dependency stalls.
            fillers = list(fillers)
            fi = 0
            onTs = []
            for pr in range(2):
                VA = avp.tile([128, QCH], F32, tag="av", name=f"VA_{qc}_{pr}")
                VB = avp.tile([128, QCH], F32, tag="av", name=f"VB_{qc}_{pr}")

                def emit_pv(kc, kk, PT):
                    st, stp = kc == 0, kc == NKC - 1
                    c0 = kc * VBLK
                    nc.tensor.matmul(
                        VA[0:65, :], vp[pr][:kk, c0 : c0 + 65],
                        PT[:kk, 0:QCH], start=st, stop=stp,
                    )
                    nc.tensor.matmul(
                        VB[0:65, :], vp[pr][:kk, c0 + 65 : c0 + VBLK],
                        PT[:kk, QCH : 2 * QCH], start=st, stop=stp,
                    )

                pend = []  # software pipeline: PV runs two chunks behind
                for kc in range(NKC):
                    if with_v and pr == 0:
                        emit_v_chunk(kc)
                    if fi < len(fillers):
                        fillers[fi]()
                        fi += 1
                    kk = 128 if kc < 16 else KREM
                    Sps = scp.tile([128, 2 * QCH], F32, tag="s", name=f"S_{qc}_{pr}_{kc}")
                    PT = ptp.tile([128, 2 * QCH], F16, tag="pt", name=f"PT_{qc}_{pr}_{kc}")
                    nc.tensor.matmul(
                        Sps[:kk, 0:QCH],
                        kT[pr][0:64, kc * 128 : kc * 128 + kk],
                        qT[pr][0:64, qc * QCH : (qc + 1) * QCH],
                        start=True, stop=True,
                    )
                    nc.tensor.matmul(
                        Sps[:kk, QCH : 2 * QCH],
                        kT[pr][64:128, kc * 128 : kc * 128 + kk],
                        qT[pr][64:128, qc * QCH : (qc + 1) * QCH],
                        start=True, stop=True,
                    )
                    # exp: one head per engine, in parallel; swap per chunk
                    if kc % 2 == 0:
                        a0, a1, d0, d1 = 0, QCH, QCH, 2 * QCH
                    else:
                        d0, d1, a0, a1 = 0, QCH, QCH, 2 * QCH
                    nc.scalar.activation(PT[:kk, a0:a1], Sps[:kk, a0:a1], AF.Exp)
                    with nc.allow_low_precision(reason="fp16 softmax weights"):
                        nc.vector.tensor_scalar(
                            PT[:kk, d0:d1].bitcast(I16),
                            Sps[:kk, d0:d1],
                            SCHRAU_A, SCHRAU_B,
                            op0=ALU.mult, op1=ALU.add,
                        )
                    pend.append((kc, kk, PT))
                    if len(pend) > 1:
                        emit_pv(*pend.pop(0))
                for p in pend:
                    emit_pv(*p)
                onT = ptp.tile([128, QCH], F16, tag="onT", name=f"onT_{qc}_{pr}")
                for hh, V in ((0, VA), (1, VB)):
                    rc = sp.tile([1, QCH], F32, tag="rc", name=f"rc_{qc}_{pr}_{hh}")
                    with nc.allow_low_precision(reason="softmax denom reciprocal"):
                        # reciprocal_approx_fast mis-reads PSUM: stage via SBUF
                        dn = sp.tile([1, QCH], F32, tag="dn", name=f"dn_{qc}_{pr}_{hh}")
                        nc.scalar.activation(dn[:], V[64:65, :], AF.Identity)
                        nc.vector.reciprocal_approx_fast(rc[:], dn[:])
                    bc = sp.tile([64, QCH], F32, tag="bc", name=f"bc_{qc}_{pr}_{hh}")
                    nc.gpsimd.partition_broadcast(bc[:], rc[:])
                    with nc.allow_low_precision(reason="fp16 attn output"):
                        nc.vector.tensor_tensor(
                            onT[hh * 64 : hh * 64 + 64, :],
                            V[0:64, :], bc[:], op=ALU.mult,
                        )
                onTs.append(onT)
            while fi < len(fillers):
                fillers[fi]()
                fi += 1
            return onTs

        emit_k([0, 1])
        emit_q([0, 1])
        emit_k([2, 3, 4])
        y_pend = None
        for qc in range(NQC):
            fillers = []
            if y_pend is not None:
                pqc, ponTs = y_pend
                for ts_ in range(4):
                    for dc in range(2):
                        fillers.extend(y_parts(pqc, ponTs, ts_, dc))
            if qc == 1:
                for qq in (2, 3):
                    for i in range(2):
                        fillers.append(lambda qq=qq, i=i: q_group(qq, i))
            onTs = att_unit(qc, with_v=(qc == 0), fillers=fillers)
            y_pend = (qc, onTs)
        emit_y(*y_pend)


def _build_program(reps=1):
    import concourse.bass as bass  # noqa: F401
    import concourse.mybir as mybir
    import concourse.tile as tile
    from concourse import bacc

    F32 = mybir.dt.float32
    F16 = mybir.dt.float16

    nc = bacc.Bacc("TRN2", num_devices=NCORES)
    dr = {
        "xT": nc.dram_tensor("xT", [D, T], F16, kind="ExternalInput"),
        "wq_p": nc.dram_tensor("wq_p", [128, 8 * GD], F16, kind="ExternalInput"),
        "wk_p": nc.dram_tensor("wk_p", [128, 8 * GD], F16, kind="ExternalInput"),
        "wv_p": nc.dram_tensor("wv_p", [128, 8 * GD], F16, kind="ExternalInput"),
        "wo_t": nc.dram_tensor("wo_t", [GD, D], F16, kind="ExternalInput"),
        "bq": nc.dram_tensor("bq", [GD, 1], F32, kind="ExternalInput"),
        "bk": nc.dram_tensor("bk", [GD, 1], F32, kind="ExternalInput"),
        "y": nc.dram_tensor("y", [S, D], F16, kind="ExternalOutput"),
    }

    with tile.TileContext(nc) as tc:
        with tc.tile_pool(name="persist", bufs=1) as pp:
            qT = [pp.tile([128, S], F16, tag=f"qT{i}", name=f"qT{i}") for i in range(2)]
            kT = [pp.tile([128, T], F16, tag=f"kT{i}", name=f"kT{i}") for i in range(2)]
            vp = [
                pp.tile([128, NKC * VBLK], F16, tag=f"vp{i}", name=f"vp{i}")
                for i in range(2)
            ]
            wo_sb = [pp.tile([128, D], F16, tag=f"wo{i}", name=f"wo{i}") for i in range(2)]
            bqt = [pp.tile([128, 1], F32, tag=f"bq{i}", name=f"bq{i}") for i in range(2)]
            bkt = [pp.tile([128, 1], F32, tag=f"bk{i}", name=f"bk{i}") for i in range(2)]
            for i in range(2):
                nc.scalar.dma_start(bqt[i][:], dr["bq"][i * 128 : (i + 1) * 128, :])
                nc.scalar.dma_start(bkt[i][:], dr["bk"][i * 128 : (i + 1) * 128, :])
            # ones columns of v' (denominator rows of the PV matmuls)
            for i in range(2):
                nc.vector.memset(vp[i][:], 1.0)
            pers = (qT, kT, vp, wo_sb, bqt, bkt)
            for _rep in range(reps):
                _emit_body(nc, tc, tile, mybir, dr, pers)
    nc.compile()
    return nc


def _get_program(reps=1):
    key = f"nc{reps}_v3"
    if key not in _prog_cache:
        _prog_cache[key] = _build_program(reps)
    return _prog_cache[key]


def _pack(a):
    # [1024, 256] -> [128, 2048]: dd-th 128-row block becomes column block dd
    return np.concatenate([a[dd * 128 : (dd + 1) * 128] for dd in range(8)], axis=1)


def _host_inputs(x, sink_tokens, wq, bq, wk, bk, wv, bv, wo, bo):
    f = np.float32
    h = np.float16
    x = np.asarray(x, f)
    sink = np.asarray(sink_tokens, f)[0]            # [NS, D]
    wq, wk, wv, wo = (np.asarray(a, f) for a in (wq, wk, wv, wo))
    bq, bk = (np.asarray(a, f) for a in (bq, bk))
    sc = np.float32(1.0 / np.sqrt(HD))
    in_maps = []
    for core in range(NCORES):
        b, g = core // 4, core % 4
        xs = np.concatenate([x[b], sink], axis=0)   # sinks LAST
        xT = np.ascontiguousarray(xs.T).astype(h)
        sl = slice(g * GD, (g + 1) * GD)
        in_maps.append({
            "xT": xT,
            "wq_p": _pack(np.ascontiguousarray(wq[sl].T) * sc).astype(h),
            "wk_p": _pack(np.ascontiguousarray(wk[sl].T)).astype(h),
            "wv_p": _pack(np.ascontiguousarray(wv[sl].T)).astype(h),
            "wo_t": np.ascontiguousarray(wo[:, sl].T).astype(h),
            "bq": (bq[sl] * sc).reshape(GD, 1).copy(),
            "bk": bk[sl].reshape(GD, 1).copy(),
        })
    return in_maps


def kernel(x, sink_tokens, wq, bq, wk, bk, wv, bv, wo, bo):
    from concourse.bass_utils import run_bass_kernel_spmd

    nc = _get_program()
    in_maps = _host_inputs(x, sink_tokens, wq, bq, wk, bk, wv, bv, wo, bo)
    res = None
    last_exc = None
    for attempt in range(3):
        try:
            res = run_bass_kernel_spmd(nc, in_maps, core_ids=list(range(NCORES)))
            break
        except Exception as e:  # transient NRT/axon failures: retry
            last_exc = e
            import time as _time
            _time.sleep(2.0 * (attempt + 1))
    if res is None:
        raise last_exc
    # host unshard: sum the 4 head-group partials per batch, then add the
    # bias constant (attn weights sum to 1 => bv passes straight through)
    bv = np.asarray(bv, np.float64)
    bo = np.asarray(bo, np.float64)
    wo64 = np.asarray(wo, np.float64)
    c = bv @ wo64.T + bo                            # [D]
    y = np.zeros((B, S, D), np.float64)
    for core in range(NCORES):
        y[core // 4] += res.results[core]["y"].astype(np.float64)
    y += c
    return y.astype(np.float32)


# revision 23
# speedup vs baseline: 1.2445x; 1.0969x over previous
# Trainium2 Bass kernel for AttentionWithSink
# B=2, S=2048, D=1024, H=16 heads (hd=64), 8 sink tokens, full bidirectional
# attention over T=2056 tokens, output projection back to D.
#
# Sharding: 8 cores = 2 batches x 4 head-groups (4 heads each).
# Each core computes QKV for its 4 heads over its batch, transposed-scores
# attention (keys on partitions => no transposes anywhere), and a partial
# output projection over its 256 head-dims. Host sums the 4 partials per
# batch (tensor-parallel unshard) and adds the bv/bo bias contribution.
#
# v3: fp16 operands end-to-end (PSUM accumulation stays fp32).
# - exp split ACT (AF.Exp) / DVE (Schraudolph int16 bitcast) by key chunk
# - no bias matmuls: bo and bv folded into a host-side constant vector
#   (softmax weights sum to 1, so  out = attn@wo.T + (bv@wo.T + bo))
# - softmax denominators: reciprocal_approx_fast directly on the PSUM row
# - input DMAs in consumption order (512-col x slices, weights interleaved)
# - y output in fp16, drains rotated ACT/DVE
import numpy as np

B, S, D, H, HD, NS = 2, 2048, 1024, 16, 64, 8
T = S + NS            # 2056 tokens incl. sinks (sinks stored LAST)
NCORES = 8
HPG = 4               # heads per group/core
GD = HPG * HD         # 256 head-dims per core
NKC = 17              # key chunks: 16*128 + 8
KREM = T - 16 * 128   # 8
NQC = 4               # query chunks
QCH = 512
VBLK = 130            # v' block per chunk: vA(64)|1|vB(64)|1

# Schraudolph fp16 exp: i16 = in*SCHRAU_A + SCHRAU_B, bitcast to fp16.
SCHRAU_A = float(2**10 / np.log(2.0))
SCHRAU_C = 0.043677448 * 2**10
SCHRAU_B = float(15 * 2**10 - SCHRAU_C)
# Each chunk's exp is split into two parallel instructions: one head on ACT
# (exact exp), the other on DVE (Schraudolph); heads swap engines per chunk
# so the Schraudolph error spreads evenly over queries.

_prog_cache = {}


def _emit_body(nc, tc, tile, mybir, dr, pers):
    F32 = mybir.dt.float32
    F16 = mybir.dt.float16
    I16 = mybir.dt.int16
    AF = mybir.ActivationFunctionType
    ALU = mybir.AluOpType
    qT, kT, vp, wo_sb, bqt, bkt = pers

    with (
        tc.tile_pool(name="xw", bufs=1) as xw,
        tc.tile_pool(name="ps", bufs=2, space="PSUM") as psb,
        tc.tile_pool(name="pt", bufs=6) as ptp,
        tc.tile_pool(name="ysb", bufs=6) as ysp,
        tc.tile_pool(name="small", bufs=4) as sp,
    ):
        scp = avp = pyp = psv = psb
        # ---------------- input DMAs (consumption order) ----------------
        xt = [xw.tile([128, T], F16, tag=f"x{dd}", name=f"x{dd}") for dd in range(8)]
        w_sb = {
            nm: xw.tile([128, 8 * GD], F16, tag=f"w{nm}", name=f"w{nm}")
            for nm in ("k", "q", "v")
        }
        # sync has a hardware DGE and no other early work: it issues everything
        # needed first (w_k, x slices 0-1, w_q). scalar (hardware DGE, but its
        # stream starts with the ~1.3us ACT table load) takes slice 2 + w_v.
        # gpsimd descriptor-gen is ~650ns per DMA (software DGE): it only gets
        # the late-needed slice 3 + w_o.
        nc.sync.dma_start(w_sb["k"][:], dr["wk_p"][:])
        def _x_dma(eng, si_c0, si_c1, dd):
            eng.dma_start(
                xt[dd][:, si_c0:si_c1], dr["xT"][dd * 128 : (dd + 1) * 128, si_c0:si_c1]
            )
        for dd in range(8):
            _x_dma(nc.sync, 0, 512, dd)
        nc.sync.dma_start(w_sb["q"][:], dr["wq_p"][:])
        for dd in range(8):
            _x_dma(nc.sync, 512, 1024, dd)
        for dd in range(8):
            _x_dma(nc.scalar, 1024, 1536, dd)
        nc.scalar.dma_start(w_sb["v"][:], dr["wv_p"][:])
        for dd in range(8):
            _x_dma(nc.gpsimd, 1536, T, dd)
        for i in range(2):
            nc.gpsimd.dma_start(wo_sb[i][:], dr["wo_t"][i * 128 : (i + 1) * 128, :])

        # ---------------- projections ----------------
        def emit_k(kc5s):
            # kT[i][gd, tok] = wk_i.T @ x ; bias+drain on ACT
            for kc5 in kc5s:
                for i in range(2):
                    n = QCH if kc5 < 4 else KREM
                    pk = psb.tile([128, QCH], F32, tag="pq", name=f"pk{i}_{kc5}")
                    for dd in range(8):
                        nc.tensor.matmul(
                            pk[:, :n],
                            w_sb["k"][:, dd * GD + i * 128 : dd * GD + i * 128 + 128],
                            xt[dd][:, kc5 * QCH : kc5 * QCH + n],
                            start=(dd == 0),
                            stop=(dd == 7),
                        )
                    nc.scalar.activation(
                        kT[i][:, kc5 * QCH : kc5 * QCH + n], pk[:, :n],
                        AF.Identity, bias=bkt[i][:, 0:1],
                    )

        def q_group(qc, i):
            pq = psb.tile([128, QCH], F32, tag="pq", name=f"pq{i}_{qc}")
            for dd in range(8):
                nc.tensor.matmul(
                    pq[:],
                    w_sb["q"][:, dd * GD + i * 128 : dd * GD + i * 128 + 128],
                    xt[dd][:, qc * QCH : (qc + 1) * QCH],
                    start=(dd == 0),
                    stop=(dd == 7),
                )
            nc.scalar.activation(
                qT[i][:, qc * QCH : (qc + 1) * QCH], pq[:],
                AF.Identity, bias=bqt[i][:, 0:1],
            )

        def emit_q(qcs):
            for qc in qcs:
                for i in range(2):
                    q_group(qc, i)

        def emit_v_chunk(tcx):
            kk = 128 if tcx < 16 else KREM
            pvt = psv.tile([128, QCH], F32, tag="pq", name=f"pv{tcx}")
            pv = pvt[:, 0:GD]
            for dd in range(8):
                nc.tensor.matmul(
                    pv[:kk, :],
                    xt[dd][:, tcx * 128 : tcx * 128 + kk],
                    w_sb["v"][:, dd * GD : (dd + 1) * GD],
                    start=(dd == 0),
                    stop=(dd == 7),
                )
            c0 = tcx * VBLK
            for pr in range(2):
                for hh in range(2):
                    h = pr * 2 + hh
                    src = pv[:kk, h * 64 : (h + 1) * 64]
                    dst = vp[pr][:kk, c0 + hh * 65 : c0 + hh * 65 + 64]
                    if (tcx + pr) % 2 == 0:
                        nc.scalar.activation(dst, src, AF.Identity)
                    else:
                        nc.vector.tensor_copy(dst, src)

        # ---------------- out-projection ----------------
        # each y unit is split into two single-matmul closures so the PE FIFO
        # can interleave them between attention chunks at fine grain
        def y_parts(qc, onTs, ts_, dc):
            py = pyp.tile([128, QCH], F32, tag="pq", name=f"py_{qc}_{ts_}_{dc}")

            def part0():
                nc.tensor.matmul(
                    py[:],
                    onTs[0][:, ts_ * 128 : (ts_ + 1) * 128],
                    wo_sb[0][:, dc * QCH : (dc + 1) * QCH],
                    start=True, stop=False,
                )

            def part1():
                nc.tensor.matmul(
                    py[:],
                    onTs[1][:, ts_ * 128 : (ts_ + 1) * 128],
                    wo_sb[1][:, dc * QCH : (dc + 1) * QCH],
                    start=False, stop=True,
                )
                ys = ysp.tile([128, QCH], F16, tag="ys", name=f"ys_{qc}_{ts_}_{dc}")
                if (ts_ + dc) % 2 == 0:
                    nc.scalar.activation(ys[:], py[:], AF.Identity)
                else:
                    nc.vector.tensor_copy(ys[:], py[:])
                nc.sync.dma_start(
                    dr["y"][qc * QCH + ts_ * 128 : qc * QCH + (ts_ + 1) * 128,
                            dc * QCH : (dc + 1) * QCH],
                    ys[:],
                )

            return [part0, part1]

        def emit_y(qc, onTs):
            for ts_ in range(4):
                for dc in range(2):
                    for p in y_parts(qc, onTs, ts_, dc):
                        p()

        # ---------------- attention ----------------
        def att_unit(qc, with_v, fillers=()):
            # fillers: independent work closures (y-proj of the previous query
            # chunk, q-proj groups) emitted INSIDE the chunk loop so the PE
            # FIFO always has ready matmuls ahead of dependency stalls.
            fillers = list(fillers)
            fi = 0
            onTs = []
            for pr in range(2):
                VA = avp.tile([128, QCH], F32, tag="av", name=f"VA_{qc}_{pr}")
                VB = avp.tile([128, QCH], F32, tag="av", name=f"VB_{qc}_{pr}")

                def emit_pv(kc, kk, PT):
                    st, stp = kc == 0, kc == NKC - 1
                    c0 = kc * VBLK
                    nc.tensor.matmul(
                        VA[0:65, :], vp[pr][:kk, c0 : c0 + 65],
                        PT[:kk, 0:QCH], start=st, stop=stp,
                    )
                    nc.tensor.matmul(
                        VB[0:65, :], vp[pr][:kk, c0 + 65 : c0 + VBLK],
                        PT[:kk, QCH : 2 * QCH], start=st, stop=stp,
                    )

                pend = []  # software pipeline: PV runs one chunk behind
                for kc in range(NKC):
                    if with_v and pr == 0:
                        emit_v_chunk(kc)
                    if fi < len(fillers):
                        fillers[fi]()
                        fi += 1
                    kk = 128 if kc < 16 else KREM
                    # per-head score tiles: one PSUM bank each (bufs=4) so the
                    # buffer-recycle slack spans two chunks
                    SpA = scp.tile([128, QCH], F32, tag="s", bufs=4,
                                   name=f"SA_{qc}_{pr}_{kc}")
                    SpB = scp.tile([128, QCH], F32, tag="s", bufs=4,
                                   name=f"SB_{qc}_{pr}_{kc}")
                    PT = ptp.tile([128, 2 * QCH], F16, tag="pt", name=f"PT_{qc}_{pr}_{kc}")
                    nc.tensor.matmul(
                        SpA[:kk, :],
                        kT[pr][0:64, kc * 128 : kc * 128 + kk],
                        qT[pr][0:64, qc * QCH : (qc + 1) * QCH],
                        start=True, stop=True,
                    )
                    nc.tensor.matmul(
                        SpB[:kk, :],
                        kT[pr][64:128, kc * 128 : kc * 128 + kk],
                        qT[pr][64:128, qc * QCH : (qc + 1) * QCH],
                        start=True, stop=True,
                    )
                    # exp: one head per engine, in parallel; swap per chunk
                    if kc % 2 == 0:
                        Sa, Sd, a0, d0 = SpA, SpB, 0, QCH
                    else:
                        Sa, Sd, a0, d0 = SpB, SpA, QCH, 0
                    nc.scalar.activation(PT[:kk, a0 : a0 + QCH], Sa[:kk, :], AF.Exp)
                    with nc.allow_low_precision(reason="fp16 softmax weights"):
                        nc.vector.tensor_scalar(
                            PT[:kk, d0 : d0 + QCH].bitcast(I16),
                            Sd[:kk, :],
                            SCHRAU_A, SCHRAU_B,
                            op0=ALU.mult, op1=ALU.add,
                        )
                    pend.append((kc, kk, PT))
                    if len(pend) > 1:
                        emit_pv(*pend.pop(0))
                for p in pend:
                    emit_pv(*p)
                onT = ptp.tile([128, QCH], F16, tag="onT", name=f"onT_{qc}_{pr}")
                for hh, V in ((0, VA), (1, VB)):
                    rc = sp.tile([1, QCH], F32, tag="rc", name=f"rc_{qc}_{pr}_{hh}")
                    with nc.allow_low_precision(reason="softmax denom reciprocal"):
                        # reciprocal_approx_fast mis-reads PSUM: stage via SBUF
                        dn = sp.tile([1, QCH], F32, tag="dn", name=f"dn_{qc}_{pr}_{hh}")
                        nc.scalar.activation(dn[:], V[64:65, :], AF.Identity)
                        nc.vector.reciprocal_approx_fast(rc[:], dn[:])
                    bc = sp.tile([64, QCH], F32, tag="bc", name=f"bc_{qc}_{pr}_{hh}")
                    nc.gpsimd.partition_broadcast(bc[:], rc[:])
                    with nc.allow_low_precision(reason="fp16 attn output"):
                        nc.vector.tensor_tensor(
                            onT[hh * 64 : hh * 64 + 64, :],
                            V[0:64, :], bc[:], op=ALU.mult,
                        )
                onTs.append(onT)
            while fi < len(fillers):
                fillers[fi]()
                fi += 1
            return onTs

        emit_k([0, 1])
        emit_q([0, 1])
        emit_k([2, 3, 4])
        y_pend = None
        for qc in range(NQC):
            fillers = []
            if y_pend is not None:
                pqc, ponTs = y_pend
                for ts_ in range(4):
                    for dc in range(2):
                        fillers.extend(y_parts(pqc, ponTs, ts_, dc))
            if qc == 1:
                for qq in (2, 3):
                    for i in range(2):
                        fillers.append(lambda qq=qq, i=i: q_group(qq, i))
            onTs = att_unit(qc, with_v=(qc == 0), fillers=fillers)
            y_pend = (qc, onTs)
        emit_y(*y_pend)


def _build_program(reps=1):
    import concourse.bass as bass  # noqa: F401
    import concourse.mybir as mybir
    import concourse.tile as tile
    from concourse import bacc

    F32 = mybir.dt.float32
    F16 = mybir.dt.float16

    nc = bacc.Bacc("TRN2", num_devices=NCORES)
    dr = {
        "xT": nc.dram_tensor("xT", [D, T], F16, kind="ExternalInput"),
        "wq_p": nc.dram_tensor("wq_p", [128, 8 * GD], F16, kind="ExternalInput"),
        "wk_p": nc.dram_tensor("wk_p", [128, 8 * GD], F16, kind="ExternalInput"),
        "wv_p": nc.dram_tensor("wv_p", [128, 8 * GD], F16, kind="ExternalInput"),
        "wo_t": nc.dram_tensor("wo_t", [GD, D], F16, kind="ExternalInput"),
        "bq": nc.dram_tensor("bq", [GD, 1], F32, kind="ExternalInput"),
        "bk": nc.dram_tensor("bk", [GD, 1], F32, kind="ExternalInput"),
        "y": nc.dram_tensor("y", [S, D], F16, kind="ExternalOutput"),
    }

    with tile.TileContext(nc) as tc:
        with tc.tile_pool(name="persist", bufs=1) as pp:
            qT = [pp.tile([128, S], F16, tag=f"qT{i}", name=f"qT{i}") for i in range(2)]
            kT = [pp.tile([128, T], F16, tag=f"kT{i}", name=f"kT{i}") for i in range(2)]
            vp = [
                pp.tile([128, NKC * VBLK], F16, tag=f"vp{i}", name=f"vp{i}")
                for i in range(2)
            ]
            wo_sb = [pp.tile([128, D], F16, tag=f"wo{i}", name=f"wo{i}") for i in range(2)]
            bqt = [pp.tile([128, 1], F32, tag=f"bq{i}", name=f"bq{i}") for i in range(2)]
            bkt = [pp.tile([128, 1], F32, tag=f"bk{i}", name=f"bk{i}") for i in range(2)]
            for i in range(2):
                nc.scalar.dma_start(bqt[i][:], dr["bq"][i * 128 : (i + 1) * 128, :])
                nc.scalar.dma_start(bkt[i][:], dr["bk"][i * 128 : (i + 1) * 128, :])
            # ones columns of v' (denominator rows of the PV matmuls)
            for i in range(2):
                nc.vector.memset(vp[i][:], 1.0)
            pers = (qT, kT, vp, wo_sb, bqt, bkt)
            for _rep in range(reps):
                _emit_body(nc, tc, tile, mybir, dr, pers)
    nc.compile()
    return nc


def _get_program(reps=1):
    key = f"nc{reps}_v3"
    if key not in _prog_cache:
        _prog_cache[key] = _build_program(reps)
    return _prog_cache[key]


def _pack(a):
    # [1024, 256] -> [128, 2048]: dd-th 128-row block becomes column block dd
    return np.concatenate([a[dd * 128 : (dd + 1) * 128] for dd in range(8)], axis=1)


def _host_inputs(x, sink_tokens, wq, bq, wk, bk, wv, bv, wo, bo):
    f = np.float32
    h = np.float16
    x = np.asarray(x, f)
    sink = np.asarray(sink_tokens, f)[0]            # [NS, D]
    wq, wk, wv, wo = (np.asarray(a, f) for a in (wq, wk, wv, wo))
    bq, bk = (np.asarray(a, f) for a in (bq, bk))
    sc = np.float32(1.0 / np.sqrt(HD))
    in_maps = []
    for core in range(NCORES):
        b, g = core // 4, core % 4
        xs = np.concatenate([x[b], sink], axis=0)   # sinks LAST
        xT = np.ascontiguousarray(xs.T).astype(h)
        sl = slice(g * GD, (g + 1) * GD)
        in_maps.append({
            "xT": xT,
            "wq_p": _pack(np.ascontiguousarray(wq[sl].T) * sc).astype(h),
            "wk_p": _pack(np.ascontiguousarray(wk[sl].T)).astype(h),
            "wv_p": _pack(np.ascontiguousarray(wv[sl].T)).astype(h),
            "wo_t": np.ascontiguousarray(wo[:, sl].T).astype(h),
            "bq": (bq[sl] * sc).reshape(GD, 1).copy(),
            "bk": bk[sl].reshape(GD, 1).copy(),
        })
    return in_maps


def kernel(x, sink_tokens, wq, bq, wk, bk, wv, bv, wo, bo):
    from concourse.bass_utils import run_bass_kernel_spmd

    nc = _get_program()
    in_maps = _host_inputs(x, sink_tokens, wq, bq, wk, bk, wv, bv, wo, bo)
    res = None
    last_exc = None
    for attempt in range(3):
        try:
            res = run_bass_kernel_spmd(nc, in_maps, core_ids=list(range(NCORES)))
            break
        except Exception as e:  # transient NRT/axon failures: retry
            last_exc = e
            import time as _time
            _time.sleep(2.0 * (attempt + 1))
    if res is None:
        raise last_exc
    # host unshard: sum the 4 head-group partials per batch, then add the
    # bias constant (attn weights sum to 1 => bv passes straight through)
    bv = np.asarray(bv, np.float64)
    bo = np.asarray(bo, np.float64)
    wo64 = np.asarray(wo, np.float64)
    c = bv @ wo64.T + bo                            # [D]
    y = np.zeros((B, S, D), np.float64)
    for core in range(NCORES):
        y[core // 4] += res.results[core]["y"].astype(np.float64)
    y += c
    return y.astype(np.float32)
